# revision 27
# baseline (speedup 1.0000x reference)
"""Trainium2 Bass kernel: causal depthwise Conv1d (K=4) + SiLU.

Reference computation (B=4, S=4096, D=2048):
    y[b, s, d] = silu( sum_k w[d, 0, k] * x[b, s-3+k, d] )   (zero-padded left)

Strategy (variant "pe"):
  * Host: transpose x to channel-major (D, B, S), left-pad each row with
    4 zeros (row length 4100), cast to bf16, shard D across the 8
    NeuronCores (256 channels each).  Depthwise conv is channel-independent
    -> no inter-core communication.
  * Core: the conv runs entirely on the TensorEngine: a matmul with a
    DIAGONAL stationary matrix diag(w[:,k]) applied to a shifted slice of
    the x tile computes w_k[d] * x[d, s+k]; the 4 taps accumulate in PSUM
    (fp32).  ACT applies native Silu reading PSUM directly, writes bf16.
  * Host: gather, transpose back, cast to f32.

Variant "dve" (fallback/AB): 4x tensor_scalar (DVE 4x mode) + 3x
tensor_tensor add + ACT shift-copy for tap alignment.
"""

import os
import sys

sys.path.insert(0, "/opt/trn_rl_repo")

import numpy as np
import ml_dtypes

N_CORES = 8
B, S, D = 4, 4096, 2048
K = 4
PAD = 4
ROW = S + PAD  # 4100
D_LOCAL = D // N_CORES  # 256
G = D_LOCAL // 128  # 2 partition groups per core

MM_N = 512  # matmul free-dim per PSUM bank
PSUM_CHUNK = 1024  # psum tile columns (2 banks), 4 bufs in flight
Y_CHUNK = 2048  # y half-tile / out-DMA granularity

VARIANT = os.environ.get("KERNEL_VARIANT", "pe")

_CACHE = {}


N_DVE_TILES = int(os.environ.get("KERNEL_N_DVE", "3"))


def _build_pe():
    """Hybrid: PE (diag-stationary matmul, PSUM accumulate) on most tiles,
    DVE full chain on N_DVE_TILES of the 8, ACT only does Silu."""
    import concourse.tile as tile
    from concourse import bacc, mybir

    nc = bacc.Bacc("TRN2", debug=False, enable_asserts=False, num_devices=N_CORES)
    bf16 = mybir.dt.bfloat16
    f32 = mybir.dt.float32

    x_ap = nc.dram_tensor("x", [G, 128, B, ROW], bf16, kind="ExternalInput").ap()
    wd_ap = nc.dram_tensor("wd", [128, G * K * 128], bf16, kind="ExternalInput").ap()
    w_ap = nc.dram_tensor("w", [128, G * K], f32, kind="ExternalInput").ap()
    out_ap = nc.dram_tensor("out", [G, 128, B, S], bf16, kind="ExternalOutput").ap()

    # spread the DVE-owned tiles through the schedule, starting early
    dve_tiles = {
        0: set(), 1: {1}, 2: {1, 4}, 3: {1, 4, 6}, 4: {1, 3, 5, 7},
    }[N_DVE_TILES]

    with tile.TileContext(nc) as tc:
        with (
            tc.tile_pool(name="wp", bufs=1) as wp,
            tc.tile_pool(name="xp", bufs=6) as xp,
            tc.tile_pool(name="tp", bufs=2) as tp,
            tc.tile_pool(name="cp", bufs=1) as cp,
            tc.tile_pool(name="ps", bufs=4, space="PSUM") as ps,
            tc.tile_pool(name="yp", bufs=4) as yp,
        ):
            # small weight DMAs first on the sync queue (~0.7us) so LDWEIGHTS
            # and the DVE tap-muls can start as soon as the first x tile lands
            wd = wp.tile([128, G * K * 128], bf16, tag="wd")
            nc.scalar.dma_start(out=wd[:], in_=wd_ap[:])
            wt = wp.tile([128, G * K], f32, tag="wt")
            nc.scalar.dma_start(out=wt[:], in_=w_ap[:])

            def wdiag(g, k):
                c0 = (g * K + k) * 128
                return wd[:, c0 : c0 + 128]

            in_dmas = []
            out_dmas = []

            def emit_dve(g, b, xt, lo, hi):
                # y[s] = sum_k w_k * xt[s + 1 + k] on the vector engine;
                # misaligned bf16 tensor_scalar measured at full 4x on HW
                W = hi - lo

                def wcol(k):
                    return wt[:, g * K + k : g * K + k + 1]

                ts = []
                for k in range(K):
                    t = tp.tile([128, W], bf16, tag=f"t{k % 2}")
                    nc.vector.tensor_scalar_mul(
                        t[:], xt[:, lo + 1 + k : lo + 1 + k + W], wcol(k)
                    )
                    ts.append(t)
                p0 = cp.tile([128, W], bf16, tag="p0")
                nc.vector.tensor_add(p0[:], ts[0][:], ts[1][:])
                p1 = cp.tile([128, W], bf16, tag="p1")
                nc.vector.tensor_add(p1[:], ts[2][:], ts[3][:])
                c = cp.tile([128, W], bf16, tag="c")
                nc.vector.tensor_add(c[:], p0[:], p1[:])
                y = yp.tile([128, W], bf16, tag="y")
                for c0 in range(0, W, PSUM_CHUNK):
                    cw = min(PSUM_CHUNK, W - c0)
                    nc.scalar.activation(
                        out=y[:, c0 : c0 + cw],
                        in_=c[:, c0 : c0 + cw],
                        func=mybir.ActivationFunctionType.Silu,
                    )
                out_dmas.append(nc.gpsimd.dma_start(
                    out=out_ap[g, :, b, lo:hi], in_=y[:],
                ))

            def emit_pe(g, b, xt, lo, hi):
                y = yp.tile([128, hi - lo], bf16, tag="y")
                for c0 in range(lo, hi, PSUM_CHUNK):
                    acc = ps.tile([128, PSUM_CHUNK], f32, tag="acc")
                    for k in range(K):
                        for n0 in range(0, PSUM_CHUNK, MM_N):
                            xlo = c0 + n0 + 1 + k
                            nc.tensor.matmul(
                                acc[:, n0 : n0 + MM_N],
                                wdiag(g, k),
                                xt[:, xlo : xlo + MM_N],
                                start=(k == 0),
                                stop=(k == K - 1),
                            )
                    nc.scalar.activation(
                        out=y[:, c0 - lo : c0 - lo + PSUM_CHUNK],
                        in_=acc[:],
                        func=mybir.ActivationFunctionType.Silu,
                    )
                out_dmas.append(nc.gpsimd.dma_start(
                    out=out_ap[g, :, b, lo:hi], in_=y[:],
                ))

            for g in range(G):
                for b in range(B):
                    tile_idx = g * B + b
                    xt = xp.tile([128, ROW], bf16, tag="xt")
                    eng = nc.sync if tile_idx < 4 else nc.scalar
                    in_dmas.append(eng.dma_start(out=xt[:], in_=x_ap[g, :, b, :]))

                    if tile_idx in dve_tiles:
                        emit_dve(g, b, xt, 0, S)
                    else:
                        emit_pe(g, b, xt, 0, S)

    nc.compile()
    return nc


def _build_dve():
    import concourse.tile as tile
    from concourse import bacc, mybir

    nc = bacc.Bacc("TRN2", debug=False, enable_asserts=False, num_devices=N_CORES)
    bf16 = mybir.dt.bfloat16
    f32 = mybir.dt.float32

    x_ap = nc.dram_tensor("x", [G, 128, B, ROW], bf16, kind="ExternalInput").ap()
    w_ap = nc.dram_tensor("w", [128, G * K], f32, kind="ExternalInput").ap()
    out_ap = nc.dram_tensor("out", [G, 128, B, S], bf16, kind="ExternalOutput").ap()

    with tile.TileContext(nc) as tc:
        with (
            tc.tile_pool(name="wp", bufs=1) as wp,
            tc.tile_pool(name="xp", bufs=2) as xp,
            tc.tile_pool(name="tp", bufs=2) as tp,
            tc.tile_pool(name="yp", bufs=2) as yp,
        ):
            wt = wp.tile([128, G * K], f32, tag="wt")
            for g in range(G):
                nc.sync.dma_start(out=wt[:, g * K : (g + 1) * K], in_=w_ap[g])

            for g in range(G):
                for b in range(B):
                    xt = xp.tile([128, ROW], bf16, tag="xt")
                    eng = nc.sync if tile_idx < 4 else nc.scalar
                    in_dmas.append(eng.dma_start(out=xt[:], in_=x_ap[g, :, b, :]))

                    xs = xp.tile([128, ROW - 1], bf16, tag="xs")
                    nc.scalar.copy(out=xs[:], in_=xt[:, 1:ROW])

                    def wcol(k):
                        return wt[:, g * K + k : g * K + k + 1]

                    t0 = tp.tile([128, S], bf16, tag="t0")
                    nc.vector.tensor_scalar_mul(t0[:], xs[:, 0:S], wcol(0))
                    t1 = tp.tile([128, S], bf16, tag="t1")
                    nc.vector.tensor_scalar_mul(t1[:], xt[:, 2 : 2 + S], wcol(1))
                    t2 = tp.tile([128, S], bf16, tag="t2")
                    nc.vector.tensor_scalar_mul(t2[:], xs[:, 2 : 2 + S], wcol(2))
                    t3 = tp.tile([128, S], bf16, tag="t3")
                    nc.vector.tensor_scalar_mul(t3[:], xt[:, 4 : 4 + S], wcol(3))

                    p0 = tp.tile([128, S], bf16, tag="p0")
                    nc.vector.tensor_add(p0[:], t0[:], t1[:])
                    p1 = tp.tile([128, S], bf16, tag="p1")
                    nc.vector.tensor_add(p1[:], t2[:], t3[:])
                    c = tp.tile([128, S], bf16, tag="c")
                    nc.vector.tensor_add(c[:], p0[:], p1[:])

                    y = yp.tile([128, S], bf16, tag="y")
                    nc.scalar.activation(
                        out=y[:], in_=c[:], func=mybir.ActivationFunctionType.Silu
                    )
                    nc.sync.dma_start(out=out_ap[g, :, b, :], in_=y[:])

    nc.compile()
    return nc


def _get_nc():
    key = "nc_" + VARIANT
    if key not in _CACHE:
        _CACHE[key] = _build_pe() if VARIANT == "pe" else _build_dve()
    return _CACHE[key]


def _make_in_maps(x, w):
    x = np.asarray(x, dtype=np.float32)
    w = np.asarray(w, dtype=np.float32)

    # (B, S, D) -> (D, B, S), bf16, left-pad rows with PAD zeros.
    x_t = np.ascontiguousarray(x.transpose(2, 0, 1)).astype(ml_dtypes.bfloat16)
    x_pad = np.zeros((D, B, ROW), dtype=ml_dtypes.bfloat16)
    x_pad[:, :, PAD:] = x_t
    w_flat = np.ascontiguousarray(w[:, 0, :])  # (D, K) f32

    in_maps = []
    for i in range(N_CORES):
        lo, hi = i * D_LOCAL, (i + 1) * D_LOCAL
        m = {"x": np.ascontiguousarray(x_pad[lo:hi].reshape(G, 128, B, ROW))}
        if VARIANT == "pe":
            m["w"] = np.ascontiguousarray(
                w_flat[lo:hi].reshape(G, 128, K).transpose(1, 0, 2).reshape(128, G * K)
            )
        else:
            m["w"] = np.ascontiguousarray(w_flat[lo:hi].reshape(G, 128, K))
        if VARIANT == "pe":
            # diag stationaries, laid out [128, G*K*128] partition-first
            wd = np.zeros((G, K, 128, 128), dtype=ml_dtypes.bfloat16)
            wl = w_flat[lo:hi].reshape(G, 128, K).astype(ml_dtypes.bfloat16)
            idx = np.arange(128)
            for g in range(G):
                for k in range(K):
                    wd[g, k, idx, idx] = wl[g, :, k]
            # (G,K,p,m) -> (p, G,K,m) -> [128, G*K*128]
            m["wd"] = np.ascontiguousarray(
                wd.transpose(2, 0, 1, 3).reshape(128, G * K * 128)
            )
        in_maps.append(m)
    return in_maps


def _assemble(results):
    parts = []
    for r in results:
        y = np.asarray(r["out"]).reshape(D_LOCAL, B, S)
        parts.append(y)
    y_full = np.concatenate(parts, axis=0)  # (D, B, S) bf16
    return np.ascontiguousarray(y_full.transpose(1, 2, 0)).astype(np.float32)


def kernel(x, w):
    from concourse.bass_utils import run_bass_kernel_spmd

    nc = _get_nc()
    in_maps = _make_in_maps(x, w)
    trace = bool(int(os.environ.get("KERNEL_TRACE", "0")))
    res = run_bass_kernel_spmd(
        nc, in_maps, core_ids=list(range(N_CORES)), trace=trace
    )
    _CACHE["last_results"] = res
    return _assemble(res.results)
